# revision 10
# baseline (speedup 1.0000x reference)
"""Trainium2 Bass kernel for DMGI-style multi-relation GCN (pos/neg convs + summaries).

Self-contained: takes full (unsharded) inputs, shards across 8 NeuronCores
(one relation per core pair, dst-tiles split within the pair), runs a Bass/Tile
kernel via run_bass_kernel_spmd, and reassembles full outputs on the host.

Algorithm per relation r (GCNConv with self-loops + symmetric normalization):
  z_pos[s] = dinv[s] * ((x*pos_mask) @ W)[s]
  z_neg[s] = dinv[s] * ((x*neg_mask)[perm] @ W)[s]   (perm folded into inputs)
  ztbl row(s) = [z_pos[s] | z_neg[s]]                (512B fp32 rows in DRAM)
  out[d]   = relu(dinv[d] * sum_{s in N(d) + self} ztbl[s] + b)

The neighbor gather uses the Ant dma_gather (int16 indices, so the table is
split into a lo region (first LO nodes) and a hi region, each with its own
zero-sentinel row at relative index 0). Per-dst slots are degree-padded inside
128-dst tiles; dst tiles are two-level degree-sorted (lo-count, then hi-count
within bands) so padding is ~7%. The segment sum is one strided DVE reduce per
tile; outputs are written sequentially and unsorted on the host.
"""

import os
import sys

sys.path.insert(0, "/opt/trn_rl_repo")

import numpy as np
import ml_dtypes

import concourse.bacc as bacc
import concourse.bass as bass
import concourse.mybir as mybir
import concourse.tile as tile
from concourse.bass_utils import run_bass_kernel_spmd

P = 128
BF16 = ml_dtypes.bfloat16

# Full problem config (matches the graded problem; overridable for small tests).
CFG = dict(N=50000, E=1600000, FI=128, FO=64, R=4, LO_TILES=248, BAND=4096,
           ZBF16=0)
RUN_MODE = os.environ.get("DMGI_RUN_MODE", "hw")  # "hw" or "sim"

_PROGRAM_CACHE = {}


def _derived(cfg):
    N = cfg["N"]
    NT = -(-N // P)           # natural tiles
    NT2 = NT + (NT % 2)       # even tile count so both cores get NT2/2 tiles
    NPAD2 = NT2 * P
    TBL = NPAD2 + P           # +1 spare tile (zero-sentinel rows live inline)
    JT = NT2 // 2             # dst tiles per core
    return NT, NT2, NPAD2, TBL, JT


def _wrap_idx(lst):
    """[num] int array -> dma_gather wrapped layout [128, num//16] int16."""
    num = len(lst)
    assert num % 16 == 0
    w16 = np.asarray(lst, np.int16).reshape(num // 16, 16).T  # [16, num/16]
    return np.tile(w16, (8, 1))


def _prep_host(cfg, x, edge_index, W, b, pos_mask, neg_mask, perm):
    """Shard/layout prep (numpy only): sorting, padding, index lists, transposes."""
    N, E, FI, FO, R = cfg["N"], cfg["E"], cfg["FI"], cfg["FO"], cfg["R"]
    NT, NT2, NPAD2, TBL, JT = _derived(cfg)
    LO = cfg["LO_TILES"] * P
    BAND = cfg["BAND"]
    assert LO + 1 <= 32767 and (NPAD2 - LO) + 1 <= 32767

    per_rel = []
    dlo_req = np.zeros((R, 2, JT), np.int64)
    dhi_req = np.zeros((R, 2, JT), np.int64)
    for r in range(R):
        src = np.asarray(edge_index[r, 0], np.int64)
        dst = np.asarray(edge_index[r, 1], np.int64)
        indeg = np.bincount(dst, minlength=N)
        sdeg = indeg + 1                       # reference degree (self-loop included)
        ids = np.arange(N)
        dst_lo_cnt = np.bincount(dst[src < LO], minlength=N)
        dst_hi_cnt = np.bincount(dst[src >= LO], minlength=N)
        locnt = dst_lo_cnt + (ids < LO)        # lo slots incl. self
        hicnt = dst_hi_cnt + (ids >= LO)
        # dummies: one lo pad slot
        locnt_f = np.concatenate([locnt, np.ones(NPAD2 - N, np.int64)])
        hicnt_f = np.concatenate([hicnt, np.zeros(NPAD2 - N, np.int64)])
        # two-level sort: by lo-count, then hi-count within bands
        order = np.argsort(locnt_f, kind="stable")
        for s in range(0, NPAD2, BAND):
            blk = order[s:s + BAND]
            order[s:s + BAND] = blk[np.argsort(hicnt_f[blk], kind="stable")]
        # edges grouped by dst, split lo/hi by src
        eorder = np.argsort(dst, kind="stable")
        src_g = src[eorder]
        lomask = src_g < LO
        src_lo = src_g[lomask]                 # lo srcs grouped by dst
        src_hi = src_g[~lomask]
        starts_lo = np.zeros(N + 1, np.int64)
        np.cumsum(dst_lo_cnt, out=starts_lo[1:])
        starts_hi = np.zeros(N + 1, np.int64)
        np.cumsum(dst_hi_cnt, out=starts_hi[1:])
        per_rel.append(dict(sdeg=sdeg, order=order, src_lo=src_lo, src_hi=src_hi,
                            starts_lo=starts_lo, starts_hi=starts_hi,
                            dst_lo_cnt=dst_lo_cnt, dst_hi_cnt=dst_hi_cnt,
                            locnt_f=locnt_f, hicnt_f=hicnt_f))
        for h in range(2):
            gi = (2 * np.arange(JT) + h)[:, None] * P + np.arange(P)[None, :]
            dsts = order[gi]
            dlo_req[r, h] = locnt_f[dsts].max(axis=1)
            dhi_req[r, h] = hicnt_f[dsts].max(axis=1)
    D_lo = dlo_req.max(axis=(0, 1))
    D_hi = dhi_req.max(axis=(0, 1))
    S = int(D_lo.sum() + D_hi.sum())
    offs = np.zeros(JT + 1, np.int64)
    np.cumsum(D_lo + D_hi, out=offs[1:])

    def padTb(a):  # [N, FI] -> bf16 [FI, NPAD2]
        out = np.zeros((FI, NPAD2), BF16)
        out[:, :N] = a.T.astype(BF16)
        return out

    xT_shared = padTb(np.asarray(x, np.float32))
    rel_arrays = []
    for r in range(R):
        pm = np.asarray(pos_mask[r], np.float32)
        nm = np.asarray(neg_mask[r], np.float32)
        pr_perm = np.asarray(perm[r], np.int64)
        xp = np.asarray(x, np.float32)[pr_perm]
        nmp = nm[pr_perm]
        biasb = np.tile(np.concatenate([np.asarray(b[r], np.float32)] * 2)[None, :],
                        (P, 1)).astype(np.float32)
        sdeg_full = np.concatenate([per_rel[r]["sdeg"],
                                    np.ones(NPAD2 - N, np.int64)])
        rel_arrays.append(dict(
            xTp=padTb(xp), pmT=padTb(pm), nmTp=padTb(nmp),
            Wm=np.asarray(W[r], np.float32).astype(BF16), biasb=biasb,
            deg_nat=sdeg_full.reshape(NT2, P).T.astype(np.float32).copy(),
            sdeg_full=sdeg_full,
        ))

    in_maps = []
    merge = []
    for c in range(8):
        r, h = c // 2, c % 2
        pr = per_rel[r]
        order = pr["order"]
        sdeg_full = rel_arrays[r]["sdeg_full"]
        gi = (2 * np.arange(JT) + h)[:, None] * P + np.arange(P)[None, :]
        dsts = order[gi]                                           # [JT, P]

        idx16 = np.zeros((P, 8 * S), np.int16)
        for jl in range(JT):
            DL, DH = int(D_lo[jl]), int(D_hi[jl])
            d = dsts[jl]
            real = d < N
            dr = np.minimum(d, N - 1)
            if DL:
                own = np.where(real & (d < LO), d + 1, 0)
                nlo = np.where(real, pr["dst_lo_cnt"][dr], 0)
                slo = pr["starts_lo"][dr]
                ar = np.arange(DL)[None, :]
                self_lo = (d < LO) & real
                pos_in = ar - self_lo[:, None]
                valid = (pos_in >= 0) & (pos_in < nlo[:, None])
                gidx = np.clip(slo[:, None] + pos_in, 0,
                               max(len(pr["src_lo"]) - 1, 0))
                vals = pr["src_lo"][gidx] + 1 if len(pr["src_lo"]) else gidx * 0
                lo_blk = np.where(valid, vals, 0)
                lo_blk[:, 0] = np.where(self_lo, own, lo_blk[:, 0])
            if DH:
                own = np.where(real & (d >= LO), d - LO + 1, 0)
                nhi = np.where(real, pr["dst_hi_cnt"][dr], 0)
                shi = pr["starts_hi"][dr]
                ar = np.arange(DH)[None, :]
                self_hi = (d >= LO) & real
                pos_in = ar - self_hi[:, None]
                valid = (pos_in >= 0) & (pos_in < nhi[:, None])
                gidx = np.clip(shi[:, None] + pos_in, 0,
                               max(len(pr["src_hi"]) - 1, 0))
                vals = (pr["src_hi"][gidx] - LO + 1) if len(pr["src_hi"]) else gidx * 0
                hi_blk = np.where(valid, vals, 0)
                hi_blk[:, 0] = np.where(self_hi, own, hi_blk[:, 0])
            o16 = 8 * int(offs[jl])
            if DL:
                idx16[:, o16:o16 + 8 * DL] = _wrap_idx(lo_blk.T.ravel())
            if DH:
                idx16[:, o16 + 8 * DL:o16 + 8 * (DL + DH)] = _wrap_idx(hi_blk.T.ravel())

        deg_sorted = sdeg_full[dsts].astype(np.float32).T.copy()   # [P, JT]
        summask = (dsts < N).astype(np.float32).T.copy()           # [P, JT]

        ra = rel_arrays[r]
        in_maps.append({
            "xT": xT_shared,
            "xTp": ra["xTp"],
            "pmT": ra["pmT"],
            "nmTp": ra["nmTp"],
            "Wm": ra["Wm"],
            "biasb": ra["biasb"],
            "deg_nat": ra["deg_nat"],
            "deg_sorted": deg_sorted,
            "summask": summask,
            "idx16": idx16,
        })
        merge.append(dsts.reshape(-1))  # output row (jl*P+p) -> original dst id
    return in_maps, merge, D_lo, D_hi, S


def _build_program(cfg, D_lo, D_hi, S, repeat=1):
    key = (tuple(sorted(cfg.items())), tuple(int(d) for d in D_lo),
           tuple(int(d) for d in D_hi), int(S), repeat)
    if key in _PROGRAM_CACHE:
        return _PROGRAM_CACHE[key]
    N, FI, FO = cfg["N"], cfg["FI"], cfg["FO"]
    NT, NT2, NPAD2, TBL, JT = _derived(cfg)
    LO = cfg["LO_TILES"] * P
    dt = mybir.dt
    f32, bf16, i16 = dt.float32, dt.bfloat16, dt.int16
    FW = 2 * FO
    zdt = bf16 if cfg.get("ZBF16") else f32

    nc = bacc.Bacc("TRN2", num_devices=8, debug=False)
    xT = nc.dram_tensor("xT", [FI, NPAD2], bf16, kind="ExternalInput")
    xTp = nc.dram_tensor("xTp", [FI, NPAD2], bf16, kind="ExternalInput")
    pmT = nc.dram_tensor("pmT", [FI, NPAD2], bf16, kind="ExternalInput")
    nmTp = nc.dram_tensor("nmTp", [FI, NPAD2], bf16, kind="ExternalInput")
    Wm = nc.dram_tensor("Wm", [FI, FO], bf16, kind="ExternalInput")
    biasb = nc.dram_tensor("biasb", [P, FW], f32, kind="ExternalInput")
    deg_nat = nc.dram_tensor("deg_nat", [P, NT2], f32, kind="ExternalInput")
    deg_sorted = nc.dram_tensor("deg_sorted", [P, JT], f32, kind="ExternalInput")
    summask = nc.dram_tensor("summask", [P, JT], f32, kind="ExternalInput")
    idx16 = nc.dram_tensor("idx16", [P, 8 * S], i16, kind="ExternalInput")

    pos_out = nc.dram_tensor("pos_out", [P, JT * FO], f32, kind="ExternalOutput")
    neg_out = nc.dram_tensor("neg_out", [P, JT * FO], f32, kind="ExternalOutput")
    sum_out = nc.dram_tensor("sum_out", [1, FO], f32, kind="ExternalOutput")

    ztbl = nc.dram_tensor("ztbl", [TBL, FW], zdt, kind="Internal")

    offs = np.zeros(JT + 1, np.int64)
    np.cumsum(np.asarray(D_lo) + np.asarray(D_hi), out=offs[1:])
    Dmax = int(max(int(D_lo[j]) + int(D_hi[j]) for j in range(JT)))

    CHT = 8                      # node tiles per phase-1 chunk
    assert NT2 % CHT == 0 and cfg["LO_TILES"] % CHT == 0
    NCH = NT2 // CHT
    LO_CH = cfg["LO_TILES"] // CHT
    OB = 49 if JT % 49 == 0 else JT  # dst tiles per output batch
    assert JT % OB == 0

    import contextlib

    with tile.TileContext(nc) as tc:
        with (
            tc.tile_pool(name="consts", bufs=1) as cpool,
            tc.tile_pool(name="ph1", bufs=2) as p1,
            tc.tile_pool(name="wb", bufs=2) as wbp,
            tc.tile_pool(name="psum1", bufs=4, space="PSUM") as pp1,
            tc.tile_pool(name="psums", bufs=1, space="PSUM") as pps,
        ):
            w_sb = cpool.tile([FI, FO], bf16, tag="w")
            nc.sync.dma_start(out=w_sb[:], in_=Wm.ap())
            bias_sb = cpool.tile([P, FW], f32, tag="bias")
            nc.sync.dma_start(out=bias_sb[:], in_=biasb.ap())
            dn_sb = cpool.tile([P, NT2], f32, tag="dn")
            nc.sync.dma_start(out=dn_sb[:], in_=deg_nat.ap())
            ds_sb = cpool.tile([P, JT], f32, tag="ds")
            nc.sync.dma_start(out=ds_sb[:], in_=deg_sorted.ap())
            sm_sb = cpool.tile([P, JT], f32, tag="sm")
            nc.sync.dma_start(out=sm_sb[:], in_=summask.ap())

            # dinv = sqrt(1/deg)
            nc.vector.reciprocal(dn_sb[:], dn_sb[:])
            nc.scalar.sqrt(dn_sb[:], dn_sb[:])
            nc.vector.reciprocal(ds_sb[:], ds_sb[:])
            nc.scalar.sqrt(ds_sb[:], ds_sb[:])

            # zero-sentinel rows (gather padding targets): rows 0 and LO+1
            zt = cpool.tile([1, FW], zdt, tag="zz")
            nc.vector.memset(zt[:], 0.0)
            nc.sync.dma_start(out=ztbl.ap()[0:1, :], in_=zt[:])
            nc.sync.dma_start(out=ztbl.ap()[LO + 1:LO + 2, :], in_=zt[:])

            rep_ctx = tc.For_i(0, repeat, 1) if repeat > 1 else contextlib.nullcontext()
            with rep_ctx:
                # ---- phase 1: masked matmuls -> scaled z table ----------------
                CW = CHT * P
                for c in range(NCH):
                    sl = slice(c * CW, (c + 1) * CW)
                    xc = p1.tile([FI, CW], bf16, tag="xc")
                    nc.sync.dma_start(out=xc[:], in_=xT.ap()[:, sl])
                    pc = p1.tile([FI, CW], bf16, tag="pc")
                    nc.sync.dma_start(out=pc[:], in_=pmT.ap()[:, sl])
                    hp = p1.tile([FI, CW], bf16, tag="hp")
                    nc.vector.tensor_tensor(out=hp[:], in0=xc[:], in1=pc[:],
                                            op=mybir.AluOpType.mult)
                    xq = p1.tile([FI, CW], bf16, tag="xq")
                    nc.sync.dma_start(out=xq[:], in_=xTp.ap()[:, sl])
                    nq = p1.tile([FI, CW], bf16, tag="nq")
                    nc.sync.dma_start(out=nq[:], in_=nmTp.ap()[:, sl])
                    hn = p1.tile([FI, CW], bf16, tag="hn")
                    nc.vector.tensor_tensor(out=hn[:], in0=xq[:], in1=nq[:],
                                            op=mybir.AluOpType.mult)

                    wb = wbp.tile([P, CHT, FW], zdt, tag="wb")
                    for t in range(CHT):
                        gt = c * CHT + t
                        tsl = slice(t * P, (t + 1) * P)
                        ps = pp1.tile([P, FO], f32, tag="ps")
                        nc.tensor.matmul(ps[:], lhsT=hp[:, tsl], rhs=w_sb[:],
                                         start=True, stop=True)
                        nc.scalar.activation(wb[:, t, 0:FO], ps[:],
                                             mybir.ActivationFunctionType.Copy,
                                             scale=dn_sb[:, gt:gt + 1])
                        ps2 = pp1.tile([P, FO], f32, tag="ps")
                        nc.tensor.matmul(ps2[:], lhsT=hn[:, tsl], rhs=w_sb[:],
                                         start=True, stop=True)
                        nc.scalar.activation(wb[:, t, FO:FW], ps2[:],
                                             mybir.ActivationFunctionType.Copy,
                                             scale=dn_sb[:, gt:gt + 1])
                    r0 = 1 + c * CW if c < LO_CH else (LO + 2) + (c - LO_CH) * CW
                    zv = ztbl.ap()[r0:r0 + CW, :].rearrange("(t p) u -> p t u", p=P)
                    nc.sync.dma_start(out=zv[:], in_=wb[:])

                # ---- phase 3: gather + segment reduce + epilogue --------------
                with (
                    tc.tile_pool(name="eidx", bufs=3) as ep,
                    tc.tile_pool(name="stg", bufs=2) as sp,
                    tc.tile_pool(name="yy", bufs=2) as yp,
                    tc.tile_pool(name="ob", bufs=2) as op_,
                ):
                    psum_sum = pps.tile([1, FO], f32, tag="acc")
                    zlo = ztbl.ap()
                    zhi = ztbl.ap()[LO + 1:, :]
                    for ob in range(JT // OB):
                        obuf = op_.tile([P, OB, FW], f32, tag="obuf")
                        for jb in range(OB):
                            j = ob * OB + jb
                            DL, DH = int(D_lo[j]), int(D_hi[j])
                            D = DL + DH
                            o16 = 8 * int(offs[j])
                            ei = ep.tile([P, 8 * Dmax], i16, tag="ei")
                            nc.sync.dma_start(out=ei[:, :8 * D],
                                              in_=idx16.ap()[:, o16:o16 + 8 * D])
                            stg = sp.tile([P, Dmax * FW], zdt, tag="stg")
                            GCH = 8   # slots per gather (<=1024 idxs: SWDGE ring cap)
                            for base, DD, zsrc in ((0, DL, zlo), (DL, DH, zhi)):
                                for k in range(0, DD, GCH):
                                    w = min(GCH, DD - k)
                                    s0 = base + k
                                    nc.gpsimd.dma_gather(
                                        out_ap=stg[:, s0 * FW:(s0 + w) * FW].rearrange(
                                            "p (g f) -> p g f", f=FW),
                                        in_ap=zsrc,
                                        idxs_ap=ei[:, 8 * s0:8 * (s0 + w)],
                                        num_idxs=P * w,
                                        num_idxs_reg=P * w,
                                        elem_size=FW,
                                    )
                            y = yp.tile([P, FW], f32, tag="y")
                            nc.vector.tensor_reduce(
                                out=y[:],
                                in_=stg[:, :D * FW].rearrange(
                                    "p (n f) -> p f n", f=FW),
                                axis=mybir.AxisListType.X,
                                op=mybir.AluOpType.add,
                            )
                            ysc = yp.tile([P, FW], f32, tag="ysc")
                            nc.scalar.activation(ysc[:], y[:],
                                                 mybir.ActivationFunctionType.Copy,
                                                 scale=ds_sb[:, j:j + 1])
                            yb = yp.tile([P, FW], f32, tag="yb")
                            nc.vector.tensor_tensor(out=yb[:], in0=ysc[:],
                                                    in1=bias_sb[:],
                                                    op=mybir.AluOpType.add)
                            nc.scalar.activation(obuf[:, jb, :], yb[:],
                                                 mybir.ActivationFunctionType.Relu)
                            nc.tensor.matmul(psum_sum[:], lhsT=sm_sb[:, j:j + 1],
                                             rhs=obuf[:, jb, 0:FO],
                                             start=(j == 0), stop=(j == JT - 1),
                                             skip_group_check=True)
                        osl = slice(ob * OB * FO, (ob + 1) * OB * FO)
                        nc.sync.dma_start(out=pos_out.ap()[:, osl],
                                          in_=obuf[:, :, 0:FO])
                        nc.sync.dma_start(out=neg_out.ap()[:, osl],
                                          in_=obuf[:, :, FO:FW])
                    ssb = yp.tile([1, FO], f32, tag="ssb")
                    nc.scalar.activation(ssb[:], psum_sum[:],
                                         mybir.ActivationFunctionType.Copy,
                                         scale=1.0 / N)
                    nc.sync.dma_start(out=sum_out.ap(), in_=ssb[:])

    nc.compile()
    _PROGRAM_CACHE[key] = nc
    return nc


def _run(nc, in_maps):
    if RUN_MODE == "sim":
        from concourse.bass_interp import CoreSim
        results = []
        for m in in_maps:
            sim = CoreSim(nc, require_finite=False, require_nnan=False)
            sim.assign_tensors(m)
            sim.simulate()
            results.append({k: sim.tensor(k).copy()
                            for k in ("pos_out", "neg_out", "sum_out")})
        return results
    res = run_bass_kernel_spmd(nc, in_maps, core_ids=list(range(8)))
    return res.results


def kernel(**inputs):
    cfg = CFG
    N, FO, R = cfg["N"], cfg["FO"], cfg["R"]
    NT, NT2, NPAD2, TBL, JT = _derived(cfg)
    x = np.asarray(inputs["x"], np.float32)
    edge_index = np.asarray(inputs["edge_index"], np.int64)
    W = np.asarray(inputs["W"], np.float32)
    b = np.asarray(inputs["b"], np.float32)
    pos_mask = np.asarray(inputs["pos_mask"], np.float32)
    neg_mask = np.asarray(inputs["neg_mask"], np.float32)
    perm = np.asarray(inputs["perm"], np.int64)

    in_maps, merge, D_lo, D_hi, S = _prep_host(cfg, x, edge_index, W, b,
                                               pos_mask, neg_mask, perm)
    nc = _build_program(cfg, D_lo, D_hi, S)
    results = _run(nc, in_maps)

    pos = np.zeros((R, N, FO), np.float32)
    neg = np.zeros((R, N, FO), np.float32)
    summ = np.zeros((R, 1, FO), np.float32)
    for c in range(8):
        r = c // 2
        ids = merge[c]                       # [JT*P] original dst ids (sorted order)
        po = results[c]["pos_out"].reshape(P, JT, FO).transpose(1, 0, 2).reshape(-1, FO)
        ne = results[c]["neg_out"].reshape(P, JT, FO).transpose(1, 0, 2).reshape(-1, FO)
        real = ids < N
        pos[r][ids[real]] = po[real]
        neg[r][ids[real]] = ne[real]
        summ[r, 0] += results[c]["sum_out"][0]
    return pos, neg, summ
